# revision 2
# baseline (speedup 1.0000x reference)
"""MHC-lite block kernel for 8x TRN2 NeuronCores.

The wall-clock of a run_bass_kernel_spmd call in this environment is
dominated by host<->device transfer of the per-call input/output maps, so
the split is chosen to minimize bytes shipped while keeping the dominant
compute (the 17 GFLOP/core FFN, 99% of model FLOPs) on the device:

  - host (f32): rms-norm stats, the three small projections (2 GFLOP),
    gates/softmax/H, layer_input (li), and the final mixed+expanded
    combine against the original f32 x (more accurate than on-device bf16).
  - device: FFN only. Each core receives its 1024 tokens' liT (bf16, 2MB)
    plus a 1/8 shard of w1/w2 (2MB); an on-device AllGather reconstructs
    the full weights, then gelu(li@W1'+b1)@W2' streams out as bf16.

Per-call traffic: ~48MB h2d + 16MB d2h (vs ~500MB for the all-device
f32-replicated-weights variant).
"""

import numpy as np
import ml_dtypes

import concourse.bacc as bacc
import concourse.mybir as mybir
import concourse.tile as tile
from concourse import bass_utils

N_CORES = 8
T_CORE = 1024          # tokens per core
NTOK = 8192            # total tokens
HID = 1024
NCH = 4096
DFF = 4096
EPS = 1.1920929e-07

F32 = mybir.dt.float32
BF16 = mybir.dt.bfloat16
BF = ml_dtypes.bfloat16

_CACHE = {}


def _build_module():
    nc = bacc.Bacc("TRN2", target_bir_lowering=False, debug=False,
                   num_devices=N_CORES)

    lit_d = nc.dram_tensor("lit", [HID, T_CORE], BF16, kind="ExternalInput").ap()
    w1s_d = nc.dram_tensor("w1s", [512, 1024], BF16, kind="ExternalInput").ap()
    w2s_d = nc.dram_tensor("w2s", [512, 1024], BF16, kind="ExternalInput").ap()
    b1_d = nc.dram_tensor("b1r", [128, 32], F32, kind="ExternalInput").ap()
    y_d = nc.dram_tensor("y", [T_CORE, HID], BF16, kind="ExternalOutput").ap()

    with tile.TileContext(nc, trace_sim=False) as tc:
        _emit(nc, tc, lit_d, w1s_d, w2s_d, b1_d, y_d)
    nc.compile()
    return nc


def _emit(nc, tc, lit_d, w1s_d, w2s_d, b1_d, y_d):
    pools = []

    def _pool(*a, **k):
        p = tc.alloc_tile_pool(*a, **k)
        pools.append(p)
        return p

    # weight shards -> bounce -> AllGather into full on-device copies
    dramp = _pool(name="dram", bufs=1, space="DRAM")
    w1b = dramp.tile([512, 1024], BF16, tag="w1b")
    w2b = dramp.tile([512, 1024], BF16, tag="w2b")
    w1g = dramp.tile([DFF, 1024], BF16, tag="w1g")
    w2g = dramp.tile([DFF, 1024], BF16, tag="w2g")
    nc.gpsimd.dma_start(w1b[:, :], w1s_d[:, :])
    nc.gpsimd.dma_start(w2b[:, :], w2s_d[:, :])
    groups = [list(range(N_CORES))]
    nc.gpsimd.collective_compute("AllGather", mybir.AluOpType.bypass,
                                 replica_groups=groups,
                                 ins=[w1b.opt()], outs=[w1g.opt()])
    nc.gpsimd.collective_compute("AllGather", mybir.AluOpType.bypass,
                                 replica_groups=groups,
                                 ins=[w2b.opt()], outs=[w2g.opt()])

    cp = _pool(name="const", bufs=1)
    w2_sb = cp.tile([128, 32 * HID], BF16, tag="w2sb")
    b1_sb = cp.tile([128, 32], F32, tag="b1")
    lit_sb = cp.tile([128, 8 * T_CORE], BF16, tag="lit")

    for m in range(32):
        nc.sync.dma_start(w2_sb[:, m * HID:(m + 1) * HID],
                          w2g[m * 128:(m + 1) * 128, :])
    nc.sync.dma_start(b1_sb[:, :], b1_d[:, :])
    for k in range(8):
        nc.sync.dma_start(lit_sb[:, k * T_CORE:(k + 1) * T_CORE],
                          lit_d[k * 128:(k + 1) * 128, :])

    w1p = _pool(name="w1", bufs=3)
    hp = _pool(name="h", bufs=4)
    outp = _pool(name="out", bufs=4)
    psA = _pool(name="psA", bufs=4, space="PSUM")
    psB = _pool(name="psB", bufs=4, space="PSUM")

    for g in range(4):                       # groups of 256 tokens
        t0 = g * 256
        fps = [[psA.tile([128, 512], F32, tag="psA",
                         name=f"fps_{g}_{ti}_{hf}")
                for hf in range(2)] for ti in range(2)]
        for m in range(32):                  # dff tiles of 128
            w1_sb = w1p.tile([128, 1024], BF16, tag="w1")
            nc.sync.dma_start(w1_sb[:, :], w1g[m * 128:(m + 1) * 128, :])
            hmp = psB.tile([128, 256], F32, tag="psB")
            for k in range(8):               # hid contraction tiles
                nc.tensor.matmul(hmp[:, :],
                                 w1_sb[:, k * 128:(k + 1) * 128],
                                 lit_sb[:, k * T_CORE + t0:
                                        k * T_CORE + t0 + 256],
                                 start=(k == 0), stop=(k == 7))
            h_m = hp.tile([128, 256], BF16, tag="h")
            nc.scalar.activation(h_m[:, :], hmp[:, :],
                                 mybir.ActivationFunctionType.Gelu_apprx_tanh,
                                 bias=b1_sb[:, m:m + 1])
            for ti in range(2):
                for hf in range(2):
                    nc.tensor.matmul(
                        fps[ti][hf][:, :],
                        h_m[:, ti * 128:(ti + 1) * 128],
                        w2_sb[:, m * HID + hf * 512: m * HID + hf * 512 + 512],
                        start=(m == 0), stop=(m == 31))
        for ti in range(2):
            for hf in range(2):
                osb = outp.tile([128, 512], BF16, tag="out")
                nc.scalar.copy(osb[:, :], fps[ti][hf][:, :])
                nc.sync.dma_start(
                    y_d[t0 + ti * 128: t0 + ti * 128 + 128,
                        hf * 512:(hf + 1) * 512],
                    osb[:, :])

    for p in reversed(pools):
        p.release()


def _sigmoid(z):
    return 0.5 * (1.0 + np.tanh(0.5 * z))


def _prep_full(x_streams, alpha_pre, alpha_post, alpha_res,
               W_pre_w, W_pre_b, W_post_w, W_post_b, W_res_w, W_res_b,
               ffn_w1, ffn_b1, ffn_w2, ffn_b2, perm_mat):
    a_pre = float(np.asarray(alpha_pre).reshape(-1)[0])
    a_post = float(np.asarray(alpha_post).reshape(-1)[0])
    a_res = float(np.asarray(alpha_res).reshape(-1)[0])

    x = np.ascontiguousarray(np.asarray(x_streams, np.float32)
                             .reshape(NTOK, NCH))
    # rms-norm scale; projections on x then scaled by s (exact reorder)
    ssq = np.einsum('tc,tc->t', x, x, dtype=np.float32)
    s = 1.0 / np.sqrt(ssq / NCH + EPS)                     # [NTOK]
    wcat = np.concatenate([np.asarray(W_pre_w, np.float32),
                           np.asarray(W_post_w, np.float32),
                           np.asarray(W_res_w, np.float32)], axis=0).T
    bcat = np.concatenate([np.asarray(W_pre_b, np.float32),
                           np.asarray(W_post_b, np.float32),
                           np.asarray(W_res_b, np.float32)])
    z = (x @ wcat) * s[:, None] + bcat                     # [NTOK, 32]
    h_pre = _sigmoid(a_pre * z[:, 0:4])                    # [NTOK, 4]
    h_post = 2.0 * _sigmoid(a_post * z[:, 4:8])            # [NTOK, 4]
    e = a_res * z[:, 8:32]
    e = np.exp(e - e.max(axis=1, keepdims=True))
    a_soft = e / e.sum(axis=1, keepdims=True)              # [NTOK, 24]
    H = a_soft @ np.asarray(perm_mat, np.float32)          # [NTOK, 16]

    x4 = x.reshape(NTOK, 4, HID)
    li = np.zeros((NTOK, HID), np.float32)
    for n in range(4):
        li += h_pre[:, n:n + 1] * x4[:, n, :]

    # device-side arrays
    w1 = np.asarray(ffn_w1, np.float32)                    # [DFF, HID]
    w1t2 = np.ascontiguousarray(
        w1.reshape(32, 128, 8, 128).transpose(0, 3, 2, 1)
        .reshape(DFF, HID)).astype(BF)                     # [m*128+p, k*128+j]
    w2t = np.ascontiguousarray(np.asarray(ffn_w2, np.float32).T).astype(BF)
    b1r = np.ascontiguousarray(
        np.asarray(ffn_b1, np.float32).reshape(32, 128).T)  # [128, 32]
    li_bf = li.astype(BF)

    in_maps = []
    for c in range(N_CORES):
        in_maps.append(dict(
            lit=np.ascontiguousarray(li_bf[c * T_CORE:(c + 1) * T_CORE].T),
            w1s=np.ascontiguousarray(w1t2[c * 512:(c + 1) * 512]),
            w2s=np.ascontiguousarray(w2t[c * 512:(c + 1) * 512]),
            b1r=b1r,
        ))
    aux = dict(x4=x4, li=li, h_post=h_post, H=H,
               b2=np.asarray(ffn_b2, np.float32))
    return in_maps, aux


def _prep_inputs(x_streams, W_pre_w, W_pre_b, W_post_w, W_post_b,
                 W_res_w, W_res_b, ffn_w1, ffn_b1, ffn_w2, ffn_b2, perm_mat):
    """Back-compat helper for test.py: in_maps only (alphas at 0.01)."""
    one = np.asarray([0.01], np.float32)
    in_maps, _ = _prep_full(x_streams, one, one, one,
                            W_pre_w, W_pre_b, W_post_w, W_post_b,
                            W_res_w, W_res_b, ffn_w1, ffn_b1, ffn_w2, ffn_b2,
                            perm_mat)
    return in_maps


def get_module():
    if "nc" not in _CACHE:
        _CACHE["nc"] = _build_module()
    return _CACHE["nc"]


def kernel(x_streams, alpha_pre, alpha_post, alpha_res,
           W_pre_w, W_pre_b, W_post_w, W_post_b, W_res_w, W_res_b,
           ffn_w1, ffn_b1, ffn_w2, ffn_b2, perm_mat):
    nc = get_module()
    in_maps, aux = _prep_full(x_streams, alpha_pre, alpha_post, alpha_res,
                              W_pre_w, W_pre_b, W_post_w, W_post_b,
                              W_res_w, W_res_b, ffn_w1, ffn_b1,
                              ffn_w2, ffn_b2, perm_mat)
    res = bass_utils.run_bass_kernel_spmd(nc, in_maps,
                                          core_ids=list(range(N_CORES)))
    ffn = np.concatenate([r["y"] for r in res.results], axis=0)
    ffn = ffn.astype(np.float32) + aux["b2"]               # [NTOK, HID]
    delta = ffn - aux["li"]
    x4, h_post, H = aux["x4"], aux["h_post"], aux["H"]
    out = np.empty((NTOK, 4, HID), np.float32)
    for i in range(4):
        acc = h_post[:, i:i + 1] * delta
        for j in range(4):
            acc += H[:, 4 * i + j:4 * i + j + 1] * x4[:, j, :]
        out[:, i, :] = acc
    return out.reshape(4, 2048, 4, 1024)


# revision 5
# speedup vs baseline: 1.8278x; 1.8278x over previous
"""MHC-lite block kernel for 8x TRN2 NeuronCores.

The wall-clock of a run_bass_kernel_spmd call in this environment is
dominated by host<->device transfer of the per-call input/output maps
(~40MB/s h2d, ~30MB/s d2h over the axon tunnel), so the split minimizes
bytes shipped while keeping the dominant compute (the 17 GFLOP/core FFN,
99% of model FLOPs) on the device:

  - host (f32): rms-norm stats, the three small projections (2 GFLOP),
    gates/softmax/H, layer_input (li), and the final mixed+expanded
    combine against the original f32 x.
  - device: FFN only, int8-in/int8-out. Each core receives its 1024
    tokens' li (int8 + per-token scale, 1MB) plus a 1/8 shard of w1/w2
    (int8, 1MB); an on-device AllGather reconstructs full weights.

Quantization scheme (all scales folded so the device needs none for
the weights): w1,w2 ship as raw int8 (global host scales s1,s2); the
w1 scale s1 is folded into the per-token li dequant scale, and s2 is
applied on the host to the returned per-token output scales. gelu runs
at true scale (psum = true preact), f32 PSUM throughout; output is
re-quantized per token with round-to-nearest + saturation.

Per-call traffic: ~18MB h2d + 8MB d2h (vs ~500MB for the all-device
f32-replicated-weights baseline).
"""

import numpy as np
import ml_dtypes

try:  # cache the XLA wrapper compile across the per-call re-jits
    import jax as _jax
    _jax.config.update("jax_compilation_cache_dir", "/tmp/jax_cache")
    _jax.config.update("jax_persistent_cache_min_compile_time_secs", 0.0)
    _jax.config.update("jax_persistent_cache_min_entry_size_bytes", -1)
except Exception:
    pass

import concourse.bacc as bacc
import concourse.mybir as mybir
import concourse.tile as tile
from concourse import bass_utils

N_CORES = 8
T_CORE = 1024          # tokens per core
NTOK = 8192            # total tokens
HID = 1024
NCH = 4096
DFF = 4096
EPS = 1.1920929e-07

F32 = mybir.dt.float32
BF16 = mybir.dt.bfloat16
I8 = mybir.dt.int8
BF = ml_dtypes.bfloat16

_CACHE = {}


def _build_module():
    nc = bacc.Bacc("TRN2", target_bir_lowering=False, debug=False,
                   num_devices=N_CORES)

    li8_d = nc.dram_tensor("li8", [T_CORE, HID], I8, kind="ExternalInput").ap()
    lsc_d = nc.dram_tensor("lsc", [T_CORE, 1], F32, kind="ExternalInput").ap()
    w1s_d = nc.dram_tensor("w1s", [512, 1024], I8, kind="ExternalInput").ap()
    w2s_d = nc.dram_tensor("w2s", [512, 1024], I8, kind="ExternalInput").ap()
    b1_d = nc.dram_tensor("b1r", [128, 32], F32, kind="ExternalInput").ap()
    y8_d = nc.dram_tensor("y8", [T_CORE, HID], I8, kind="ExternalOutput").ap()
    ysc_d = nc.dram_tensor("ysc", [T_CORE, 1], F32, kind="ExternalOutput").ap()

    with tile.TileContext(nc, trace_sim=False) as tc:
        _emit(nc, tc, li8_d, lsc_d, w1s_d, w2s_d, b1_d, y8_d, ysc_d)
    nc.compile()
    return nc


def _emit(nc, tc, li8_d, lsc_d, w1s_d, w2s_d, b1_d, y8_d, ysc_d):
    pools = []

    def _pool(*a, **k):
        p = tc.alloc_tile_pool(*a, **k)
        pools.append(p)
        return p

    # weight shards -> bounce -> AllGather into full on-device copies
    dramp = _pool(name="dram", bufs=1, space="DRAM")
    w1b = dramp.tile([512, 1024], I8, tag="w1b")
    w2b = dramp.tile([512, 1024], I8, tag="w2b")
    w1g = dramp.tile([DFF, 1024], I8, tag="w1g")
    w2g = dramp.tile([DFF, 1024], I8, tag="w2g")
    nc.gpsimd.dma_start(w1b[:, :], w1s_d[:, :])
    nc.gpsimd.dma_start(w2b[:, :], w2s_d[:, :])
    groups = [list(range(N_CORES))]
    nc.gpsimd.collective_compute("AllGather", mybir.AluOpType.bypass,
                                 replica_groups=groups,
                                 ins=[w1b.opt()], outs=[w1g.opt()])
    nc.gpsimd.collective_compute("AllGather", mybir.AluOpType.bypass,
                                 replica_groups=groups,
                                 ins=[w2b.opt()], outs=[w2g.opt()])

    cp = _pool(name="const", bufs=1)
    w1_sb = cp.tile([128, 32 * HID], BF16, tag="w1sb")   # raw int values
    w2_sb = cp.tile([128, 32 * HID], BF16, tag="w2sb")   # raw int values
    b1_sb = cp.tile([128, 32], F32, tag="b1")
    lit_sb = cp.tile([128, 8 * T_CORE], BF16, tag="lit")  # li^T, s1 folded
    lsc_sb = cp.tile([128, 8], F32, tag="lsc")

    stp = _pool(name="stage", bufs=3)
    for m in range(32):
        st1 = stp.tile([128, HID], I8, tag="w1st")
        nc.sync.dma_start(st1[:, :], w1g[m * 128:(m + 1) * 128, :])
        nc.vector.tensor_copy(w1_sb[:, m * HID:(m + 1) * HID], st1[:, :])
        st2 = stp.tile([128, HID], I8, tag="w2st")
        nc.sync.dma_start(st2[:, :], w2g[m * 128:(m + 1) * 128, :])
        nc.vector.tensor_copy(w2_sb[:, m * HID:(m + 1) * HID], st2[:, :])
    nc.sync.dma_start(b1_sb[:, :], b1_d[:, :])
    nc.sync.dma_start(lsc_sb[:, :],
                      lsc_d.rearrange("(r p) o -> p (r o)", p=128))

    # li: load int8 token-major, dequant (scale includes s1), transpose
    lqp = _pool(name="lq", bufs=3)
    lbp = _pool(name="lb", bufs=3)
    for r in range(8):                       # token tiles
        li8 = lqp.tile([128, HID], I8, tag="li8")
        nc.sync.dma_start(li8[:, :], li8_d[r * 128:(r + 1) * 128, :])
        libf = lbp.tile([128, HID], BF16, tag="libf")
        nc.vector.tensor_scalar_mul(libf[:, :], li8[:, :],
                                    lsc_sb[:, r:r + 1])
        for k in range(8):                   # hid tiles -> transpose
            nc.sync.dma_start_transpose(
                lit_sb[:, k * T_CORE + r * 128: k * T_CORE + r * 128 + 128],
                libf[:, k * 128:(k + 1) * 128])

    hp = _pool(name="h", bufs=4)
    qp = _pool(name="q", bufs=4)
    scp = _pool(name="sc", bufs=4)
    psA = _pool(name="psA", bufs=4, space="PSUM")
    psB = _pool(name="psB", bufs=4, space="PSUM")

    for g in range(4):                       # groups of 256 tokens
        t0 = g * 256
        fps = [[psA.tile([128, 512], F32, tag="psA",
                         name=f"fps_{g}_{ti}_{hf}")
                for hf in range(2)] for ti in range(2)]
        for m in range(32):                  # dff tiles of 128
            hmp = psB.tile([128, 256], F32, tag="psB")
            for k in range(8):               # hid contraction tiles
                nc.tensor.matmul(hmp[:, :],
                                 w1_sb[:, m * HID + k * 128:
                                       m * HID + k * 128 + 128],
                                 lit_sb[:, k * T_CORE + t0:
                                        k * T_CORE + t0 + 256],
                                 start=(k == 0), stop=(k == 7))
            h_m = hp.tile([128, 256], BF16, tag="h")
            nc.scalar.activation(h_m[:, :], hmp[:, :],
                                 mybir.ActivationFunctionType.Gelu_apprx_tanh,
                                 bias=b1_sb[:, m:m + 1])
            for ti in range(2):
                for hf in range(2):
                    nc.tensor.matmul(
                        fps[ti][hf][:, :],
                        h_m[:, ti * 128:(ti + 1) * 128],
                        w2_sb[:, m * HID + hf * 512: m * HID + hf * 512 + 512],
                        start=(m == 0), stop=(m == 31))
        for ti in range(2):
            # per-token absmax over the 1024 output cols (raw 1/s2 scale)
            sc = scp.tile([128, 8], F32, tag="sc")
            nc.vector.reduce_max(sc[:, 0:1], fps[ti][0][:, :],
                                 axis=mybir.AxisListType.X,
                                 apply_absolute_value=True)
            nc.vector.reduce_max(sc[:, 1:2], fps[ti][1][:, :],
                                 axis=mybir.AxisListType.X,
                                 apply_absolute_value=True)
            nc.vector.tensor_max(sc[:, 2:3], sc[:, 0:1], sc[:, 1:2])
            nc.scalar.activation(sc[:, 3:4], sc[:, 2:3],
                                 mybir.ActivationFunctionType.Copy,
                                 bias=1e-12)
            nc.vector.reciprocal(sc[:, 4:5], sc[:, 3:4])
            nc.scalar.activation(sc[:, 5:6], sc[:, 4:5],
                                 mybir.ActivationFunctionType.Copy,
                                 scale=127.0)                  # inv
            nc.scalar.activation(sc[:, 6:7], sc[:, 3:4],
                                 mybir.ActivationFunctionType.Copy,
                                 scale=1.0 / 127.0)            # out scale
            nc.sync.dma_start(ysc_d[t0 + ti * 128: t0 + ti * 128 + 128, :],
                              sc[:, 6:7])
            for hf in range(2):
                q = qp.tile([128, 512], I8, tag="q")
                nc.vector.tensor_scalar_mul(q[:, :], fps[ti][hf][:, :],
                                            sc[:, 5:6])
                nc.sync.dma_start(
                    y8_d[t0 + ti * 128: t0 + ti * 128 + 128,
                         hf * 512:(hf + 1) * 512],
                    q[:, :])

    for p in reversed(pools):
        p.release()


def _sigmoid(z):
    return 0.5 * (1.0 + np.tanh(0.5 * z))


def _prep_full(x_streams, alpha_pre, alpha_post, alpha_res,
               W_pre_w, W_pre_b, W_post_w, W_post_b, W_res_w, W_res_b,
               ffn_w1, ffn_b1, ffn_w2, ffn_b2, perm_mat):
    a_pre = float(np.asarray(alpha_pre).reshape(-1)[0])
    a_post = float(np.asarray(alpha_post).reshape(-1)[0])
    a_res = float(np.asarray(alpha_res).reshape(-1)[0])

    x = np.ascontiguousarray(np.asarray(x_streams, np.float32)
                             .reshape(NTOK, NCH))
    # rms-norm scale; projections on x then scaled by s (exact reorder)
    ssq = np.einsum('tc,tc->t', x, x, dtype=np.float32)
    s = 1.0 / np.sqrt(ssq / NCH + EPS)                     # [NTOK]
    wcat = np.concatenate([np.asarray(W_pre_w, np.float32),
                           np.asarray(W_post_w, np.float32),
                           np.asarray(W_res_w, np.float32)], axis=0).T
    bcat = np.concatenate([np.asarray(W_pre_b, np.float32),
                           np.asarray(W_post_b, np.float32),
                           np.asarray(W_res_b, np.float32)])
    z = (x @ wcat) * s[:, None] + bcat                     # [NTOK, 32]
    h_pre = _sigmoid(a_pre * z[:, 0:4])                    # [NTOK, 4]
    h_post = 2.0 * _sigmoid(a_post * z[:, 4:8])            # [NTOK, 4]
    e = a_res * z[:, 8:32]
    e = np.exp(e - e.max(axis=1, keepdims=True))
    a_soft = e / e.sum(axis=1, keepdims=True)              # [NTOK, 24]
    H = a_soft @ np.asarray(perm_mat, np.float32)          # [NTOK, 16]

    x4 = x.reshape(NTOK, 4, HID)
    li = np.zeros((NTOK, HID), np.float32)
    for n in range(4):
        li += h_pre[:, n:n + 1] * x4[:, n, :]

    # int8 quantization (host): weights global-scale, li per-token scale
    w1 = np.asarray(ffn_w1, np.float32)                    # [DFF, HID]
    w1t2 = np.ascontiguousarray(
        w1.reshape(32, 128, 8, 128).transpose(0, 3, 2, 1)
        .reshape(DFF, HID))                                # [m*128+p, k*128+j]
    s1 = np.abs(w1t2).max() / 127.0
    w1q = np.clip(np.round(w1t2 / s1), -127, 127).astype(np.int8)
    w2t = np.ascontiguousarray(np.asarray(ffn_w2, np.float32).T)
    s2 = np.abs(w2t).max() / 127.0
    w2q = np.clip(np.round(w2t / s2), -127, 127).astype(np.int8)
    b1r = np.ascontiguousarray(
        np.asarray(ffn_b1, np.float32).reshape(32, 128).T)  # [128, 32]

    rowmax = np.abs(li).max(axis=1) + 1e-30                # [NTOK]
    li_q = np.clip(np.round(li * (127.0 / rowmax[:, None])),
                   -127, 127).astype(np.int8)
    lsc = ((rowmax / 127.0) * s1).astype(np.float32)[:, None]  # fold s1

    in_maps = []
    for c in range(N_CORES):
        sl = slice(c * T_CORE, (c + 1) * T_CORE)
        in_maps.append(dict(
            li8=np.ascontiguousarray(li_q[sl]),
            lsc=np.ascontiguousarray(lsc[sl]),
            w1s=np.ascontiguousarray(w1q[c * 512:(c + 1) * 512]),
            w2s=np.ascontiguousarray(w2q[c * 512:(c + 1) * 512]),
            b1r=b1r,
        ))
    aux = dict(x4=x4, li=li, h_post=h_post, H=H, s2=s2,
               b2=np.asarray(ffn_b2, np.float32))
    return in_maps, aux


def _prep_inputs(x_streams, W_pre_w, W_pre_b, W_post_w, W_post_b,
                 W_res_w, W_res_b, ffn_w1, ffn_b1, ffn_w2, ffn_b2, perm_mat):
    """Back-compat helper for test.py: in_maps only (alphas at 0.01)."""
    one = np.asarray([0.01], np.float32)
    in_maps, _ = _prep_full(x_streams, one, one, one,
                            W_pre_w, W_pre_b, W_post_w, W_post_b,
                            W_res_w, W_res_b, ffn_w1, ffn_b1, ffn_w2, ffn_b2,
                            perm_mat)
    return in_maps


def get_module():
    if "nc" not in _CACHE:
        _CACHE["nc"] = _build_module()
    return _CACHE["nc"]


def kernel(x_streams, alpha_pre, alpha_post, alpha_res,
           W_pre_w, W_pre_b, W_post_w, W_post_b, W_res_w, W_res_b,
           ffn_w1, ffn_b1, ffn_w2, ffn_b2, perm_mat):
    nc = get_module()
    in_maps, aux = _prep_full(x_streams, alpha_pre, alpha_post, alpha_res,
                              W_pre_w, W_pre_b, W_post_w, W_post_b,
                              W_res_w, W_res_b, ffn_w1, ffn_b1,
                              ffn_w2, ffn_b2, perm_mat)
    res = bass_utils.run_bass_kernel_spmd(nc, in_maps,
                                          core_ids=list(range(N_CORES)))
    q = np.concatenate([r["y8"] for r in res.results], axis=0)
    ysc = np.concatenate([r["ysc"] for r in res.results], axis=0)
    ffn = q.astype(np.float32) * (ysc * aux["s2"]) + aux["b2"]
    delta = ffn - aux["li"]
    x4, h_post, H = aux["x4"], aux["h_post"], aux["H"]
    out = np.empty((NTOK, 4, HID), np.float32)
    for i in range(4):
        acc = h_post[:, i:i + 1] * delta
        for j in range(4):
            acc += H[:, 4 * i + j:4 * i + j + 1] * x4[:, j, :]
        out[:, i, :] = acc
    return out.reshape(4, 2048, 4, 1024)


# revision 6
# speedup vs baseline: 2.0303x; 1.1108x over previous
"""MHC-lite block kernel for 8x TRN2 NeuronCores.

The wall-clock of a run_bass_kernel_spmd call in this environment is
dominated by host<->device transfer of the per-call input/output maps
(~40MB/s h2d, ~30MB/s d2h over the axon tunnel, plus ~5ms fixed cost per
tensor-shard), so the split minimizes bytes AND tensor count while
keeping the dominant compute (the 17 GFLOP/core FFN, 99% of model FLOPs)
on the device:

  - host (f32): rms-norm stats, the three small projections (2 GFLOP),
    gates/softmax/H, layer_input (li), and the final mixed+expanded
    combine against the original f32 x.
  - device: FFN only, int8-in/int8-out. Each core receives its 1024
    tokens' li (int8 + per-token scale) plus a 1/8 shard of w1/w2
    (int8); an on-device AllGather reconstructs full weights.

Quantization scheme (scales folded so the device needs none for the
weights): w1,w2 ship as raw int8 (global host scales s1,s2); s1 is
folded into the per-token li dequant scale, s2 is applied on the host
to the returned per-token output scales. gelu runs at true scale, f32
PSUM throughout; output is re-quantized per token with round-to-nearest
(verified RNE + saturation on HW). The f32 output scales ride in the
last 4 rows of the int8 output tensor via a bitcast AP.

Per-call traffic: ~18MB h2d + 8.4MB d2h in 3 tensors (vs ~500MB/10
tensors for the all-device f32-replicated-weights baseline).
"""

import numpy as np
import ml_dtypes

try:  # cache the XLA wrapper compile across the per-call re-jits
    import jax as _jax
    _jax.config.update("jax_compilation_cache_dir", "/tmp/jax_cache")
    _jax.config.update("jax_persistent_cache_min_compile_time_secs", 0.0)
    _jax.config.update("jax_persistent_cache_min_entry_size_bytes", -1)
except Exception:
    pass

import concourse.bacc as bacc
import concourse.mybir as mybir
import concourse.tile as tile
from concourse import bass_utils

N_CORES = 8
T_CORE = 1024          # tokens per core
NTOK = 8192            # total tokens
HID = 1024
NCH = 4096
DFF = 4096
EPS = 1.1920929e-07

F32 = mybir.dt.float32
BF16 = mybir.dt.bfloat16
I8 = mybir.dt.int8
BF = ml_dtypes.bfloat16

_CACHE = {}


def _build_module():
    nc = bacc.Bacc("TRN2", target_bir_lowering=False, debug=False,
                   num_devices=N_CORES)

    # blob8 rows: [0:1024] li8 (token-major), [1024:1536] w1 shard,
    # [1536:2048] w2 shard.  blobf cols: [0:8] per-token li scales in
    # SBUF layout (s1 folded), [8:40] b1 as [128, 32].
    blob8_d = nc.dram_tensor("blob8", [2048, 1024], I8,
                             kind="ExternalInput").ap()
    blobf_d = nc.dram_tensor("blobf", [128, 40], F32,
                             kind="ExternalInput").ap()
    # y8 rows [0:1024]: int8 ffn out; rows [1024:1028]: per-token f32
    # scales (maxabs/127, raw 1/s2 units) bitcast to int8 bytes.
    y8_d = nc.dram_tensor("y8", [1028, 1024], I8, kind="ExternalOutput").ap()

    with tile.TileContext(nc, trace_sim=False) as tc:
        _emit(nc, tc, blob8_d, blobf_d, y8_d)
    nc.compile()
    return nc


def _emit(nc, tc, blob8_d, blobf_d, y8_d):
    pools = []

    def _pool(*a, **k):
        p = tc.alloc_tile_pool(*a, **k)
        pools.append(p)
        return p

    ysc_view = y8_d[1024:1028, :].bitcast(F32) \
        .rearrange("a (t o) -> (a t) o", t=256)          # [1024, 1] f32

    # weight shards -> bounce -> AllGather into full on-device copies
    dramp = _pool(name="dram", bufs=1, space="DRAM")
    w1b = dramp.tile([512, 1024], I8, tag="w1b")
    w2b = dramp.tile([512, 1024], I8, tag="w2b")
    w1g = dramp.tile([DFF, 1024], I8, tag="w1g")
    w2g = dramp.tile([DFF, 1024], I8, tag="w2g")
    nc.gpsimd.dma_start(w1b[:, :], blob8_d[1024:1536, :])
    nc.gpsimd.dma_start(w2b[:, :], blob8_d[1536:2048, :])
    groups = [list(range(N_CORES))]
    nc.gpsimd.collective_compute("AllGather", mybir.AluOpType.bypass,
                                 replica_groups=groups,
                                 ins=[w1b.opt()], outs=[w1g.opt()])
    nc.gpsimd.collective_compute("AllGather", mybir.AluOpType.bypass,
                                 replica_groups=groups,
                                 ins=[w2b.opt()], outs=[w2g.opt()])

    cp = _pool(name="const", bufs=1)
    w1_sb = cp.tile([128, 32 * HID], BF16, tag="w1sb")   # raw int values
    w2_sb = cp.tile([128, 32 * HID], BF16, tag="w2sb")   # raw int values
    f_sb = cp.tile([128, 40], F32, tag="fsb")            # lsc | b1
    lit_sb = cp.tile([128, 8 * T_CORE], BF16, tag="lit")  # li^T, s1 folded

    nc.sync.dma_start(f_sb[:, :], blobf_d[:, :])
    lsc_sb = f_sb[:, 0:8]
    b1_sb = f_sb[:, 8:40]

    stp = _pool(name="stage", bufs=3)
    for m in range(32):
        st1 = stp.tile([128, HID], I8, tag="w1st")
        nc.sync.dma_start(st1[:, :], w1g[m * 128:(m + 1) * 128, :])
        nc.vector.tensor_copy(w1_sb[:, m * HID:(m + 1) * HID], st1[:, :])
        st2 = stp.tile([128, HID], I8, tag="w2st")
        nc.sync.dma_start(st2[:, :], w2g[m * 128:(m + 1) * 128, :])
        nc.vector.tensor_copy(w2_sb[:, m * HID:(m + 1) * HID], st2[:, :])

    # li: load int8 token-major, dequant (scale includes s1), transpose
    lqp = _pool(name="lq", bufs=3)
    lbp = _pool(name="lb", bufs=3)
    for r in range(8):                       # token tiles
        li8 = lqp.tile([128, HID], I8, tag="li8")
        nc.sync.dma_start(li8[:, :], blob8_d[r * 128:(r + 1) * 128, :])
        libf = lbp.tile([128, HID], BF16, tag="libf")
        nc.vector.tensor_scalar_mul(libf[:, :], li8[:, :],
                                    lsc_sb[:, r:r + 1])
        for k in range(8):                   # hid tiles -> transpose
            nc.sync.dma_start_transpose(
                lit_sb[:, k * T_CORE + r * 128: k * T_CORE + r * 128 + 128],
                libf[:, k * 128:(k + 1) * 128])

    hp = _pool(name="h", bufs=4)
    qp = _pool(name="q", bufs=4)
    scp = _pool(name="sc", bufs=4)
    psA = _pool(name="psA", bufs=4, space="PSUM")
    psB = _pool(name="psB", bufs=4, space="PSUM")

    for g in range(4):                       # groups of 256 tokens
        t0 = g * 256
        fps = [[psA.tile([128, 512], F32, tag="psA",
                         name=f"fps_{g}_{ti}_{hf}")
                for hf in range(2)] for ti in range(2)]
        for m in range(32):                  # dff tiles of 128
            hmp = psB.tile([128, 256], F32, tag="psB")
            for k in range(8):               # hid contraction tiles
                nc.tensor.matmul(hmp[:, :],
                                 w1_sb[:, m * HID + k * 128:
                                       m * HID + k * 128 + 128],
                                 lit_sb[:, k * T_CORE + t0:
                                        k * T_CORE + t0 + 256],
                                 start=(k == 0), stop=(k == 7))
            h_m = hp.tile([128, 256], BF16, tag="h")
            nc.scalar.activation(h_m[:, :], hmp[:, :],
                                 mybir.ActivationFunctionType.Gelu_apprx_tanh,
                                 bias=b1_sb[:, m:m + 1])
            for ti in range(2):
                for hf in range(2):
                    nc.tensor.matmul(
                        fps[ti][hf][:, :],
                        h_m[:, ti * 128:(ti + 1) * 128],
                        w2_sb[:, m * HID + hf * 512: m * HID + hf * 512 + 512],
                        start=(m == 0), stop=(m == 31))
        for ti in range(2):
            # per-token absmax over the 1024 output cols (raw 1/s2 scale)
            sc = scp.tile([128, 8], F32, tag="sc")
            nc.vector.reduce_max(sc[:, 0:1], fps[ti][0][:, :],
                                 axis=mybir.AxisListType.X,
                                 apply_absolute_value=True)
            nc.vector.reduce_max(sc[:, 1:2], fps[ti][1][:, :],
                                 axis=mybir.AxisListType.X,
                                 apply_absolute_value=True)
            nc.vector.tensor_max(sc[:, 2:3], sc[:, 0:1], sc[:, 1:2])
            nc.scalar.activation(sc[:, 3:4], sc[:, 2:3],
                                 mybir.ActivationFunctionType.Copy,
                                 bias=1e-12)
            nc.vector.reciprocal(sc[:, 4:5], sc[:, 3:4])
            nc.scalar.activation(sc[:, 5:6], sc[:, 4:5],
                                 mybir.ActivationFunctionType.Copy,
                                 scale=127.0)                  # inv
            nc.scalar.activation(sc[:, 6:7], sc[:, 3:4],
                                 mybir.ActivationFunctionType.Copy,
                                 scale=1.0 / 127.0)            # out scale
            nc.sync.dma_start(ysc_view[t0 + ti * 128: t0 + ti * 128 + 128, :],
                              sc[:, 6:7])
            for hf in range(2):
                q = qp.tile([128, 512], I8, tag="q")
                nc.vector.tensor_scalar_mul(q[:, :], fps[ti][hf][:, :],
                                            sc[:, 5:6])
                nc.sync.dma_start(
                    y8_d[t0 + ti * 128: t0 + ti * 128 + 128,
                         hf * 512:(hf + 1) * 512],
                    q[:, :])

    for p in reversed(pools):
        p.release()


def _sigmoid(z):
    return 0.5 * (1.0 + np.tanh(0.5 * z))


def _prep_full(x_streams, alpha_pre, alpha_post, alpha_res,
               W_pre_w, W_pre_b, W_post_w, W_post_b, W_res_w, W_res_b,
               ffn_w1, ffn_b1, ffn_w2, ffn_b2, perm_mat):
    a_pre = float(np.asarray(alpha_pre).reshape(-1)[0])
    a_post = float(np.asarray(alpha_post).reshape(-1)[0])
    a_res = float(np.asarray(alpha_res).reshape(-1)[0])

    x = np.ascontiguousarray(np.asarray(x_streams, np.float32)
                             .reshape(NTOK, NCH))
    # rms-norm scale; projections on x then scaled by s (exact reorder)
    ssq = np.einsum('tc,tc->t', x, x, dtype=np.float32)
    s = 1.0 / np.sqrt(ssq / NCH + EPS)                     # [NTOK]
    wcat = np.concatenate([np.asarray(W_pre_w, np.float32),
                           np.asarray(W_post_w, np.float32),
                           np.asarray(W_res_w, np.float32)], axis=0).T
    bcat = np.concatenate([np.asarray(W_pre_b, np.float32),
                           np.asarray(W_post_b, np.float32),
                           np.asarray(W_res_b, np.float32)])
    z = (x @ wcat) * s[:, None] + bcat                     # [NTOK, 32]
    h_pre = _sigmoid(a_pre * z[:, 0:4])                    # [NTOK, 4]
    h_post = 2.0 * _sigmoid(a_post * z[:, 4:8])            # [NTOK, 4]
    e = a_res * z[:, 8:32]
    e = np.exp(e - e.max(axis=1, keepdims=True))
    a_soft = e / e.sum(axis=1, keepdims=True)              # [NTOK, 24]
    H = a_soft @ np.asarray(perm_mat, np.float32)          # [NTOK, 16]

    x4 = x.reshape(NTOK, 4, HID)
    li = np.zeros((NTOK, HID), np.float32)
    for n in range(4):
        li += h_pre[:, n:n + 1] * x4[:, n, :]

    # int8 quantization (host): weights global-scale, li per-token scale
    w1 = np.asarray(ffn_w1, np.float32)                    # [DFF, HID]
    w1t2 = np.ascontiguousarray(
        w1.reshape(32, 128, 8, 128).transpose(0, 3, 2, 1)
        .reshape(DFF, HID))                                # [m*128+p, k*128+j]
    s1 = np.abs(w1t2).max() / 127.0
    w1q = np.clip(np.round(w1t2 / s1), -127, 127).astype(np.int8)
    w2t = np.ascontiguousarray(np.asarray(ffn_w2, np.float32).T)
    s2 = np.abs(w2t).max() / 127.0
    w2q = np.clip(np.round(w2t / s2), -127, 127).astype(np.int8)
    b1r = np.ascontiguousarray(
        np.asarray(ffn_b1, np.float32).reshape(32, 128).T)  # [128, 32]

    rowmax = np.abs(li).max(axis=1) + 1e-30                # [NTOK]
    li_q = np.clip(np.round(li * (127.0 / rowmax[:, None])),
                   -127, 127).astype(np.int8)
    lsc = ((rowmax / 127.0) * s1).astype(np.float32)       # fold s1

    in_maps = []
    for c in range(N_CORES):
        sl = slice(c * T_CORE, (c + 1) * T_CORE)
        blob8 = np.empty((2048, 1024), np.int8)
        blob8[0:1024] = li_q[sl]
        blob8[1024:1536] = w1q[c * 512:(c + 1) * 512]
        blob8[1536:2048] = w2q[c * 512:(c + 1) * 512]
        blobf = np.empty((128, 40), np.float32)
        blobf[:, 0:8] = lsc[sl].reshape(8, 128).T          # [p, r]
        blobf[:, 8:40] = b1r
        in_maps.append(dict(blob8=blob8, blobf=blobf))
    aux = dict(x4=x4, li=li, h_post=h_post, H=H, s2=s2,
               b2=np.asarray(ffn_b2, np.float32))
    return in_maps, aux


def _prep_inputs(x_streams, W_pre_w, W_pre_b, W_post_w, W_post_b,
                 W_res_w, W_res_b, ffn_w1, ffn_b1, ffn_w2, ffn_b2, perm_mat):
    """Back-compat helper for test.py: in_maps only (alphas at 0.01)."""
    one = np.asarray([0.01], np.float32)
    in_maps, _ = _prep_full(x_streams, one, one, one,
                            W_pre_w, W_pre_b, W_post_w, W_post_b,
                            W_res_w, W_res_b, ffn_w1, ffn_b1, ffn_w2, ffn_b2,
                            perm_mat)
    return in_maps


def get_module():
    if "nc" not in _CACHE:
        _CACHE["nc"] = _build_module()
    return _CACHE["nc"]


def kernel(x_streams, alpha_pre, alpha_post, alpha_res,
           W_pre_w, W_pre_b, W_post_w, W_post_b, W_res_w, W_res_b,
           ffn_w1, ffn_b1, ffn_w2, ffn_b2, perm_mat):
    nc = get_module()
    in_maps, aux = _prep_full(x_streams, alpha_pre, alpha_post, alpha_res,
                              W_pre_w, W_pre_b, W_post_w, W_post_b,
                              W_res_w, W_res_b, ffn_w1, ffn_b1,
                              ffn_w2, ffn_b2, perm_mat)
    res = bass_utils.run_bass_kernel_spmd(nc, in_maps,
                                          core_ids=list(range(N_CORES)))
    qs, scs = [], []
    for r in res.results:
        y8 = r["y8"]
        qs.append(y8[0:1024])
        scs.append(y8[1024:1028].reshape(-1).view(np.float32))
    q = np.concatenate(qs, axis=0)
    ysc = np.concatenate(scs, axis=0)[:, None]             # [NTOK, 1]
    ffn = q.astype(np.float32) * (ysc * aux["s2"]) + aux["b2"]
    delta = ffn - aux["li"]
    x4, h_post, H = aux["x4"], aux["h_post"], aux["H"]
    out = np.empty((NTOK, 4, HID), np.float32)
    for i in range(4):
        acc = h_post[:, i:i + 1] * delta
        for j in range(4):
            acc += H[:, 4 * i + j:4 * i + j + 1] * x4[:, j, :]
        out[:, i, :] = acc
    return out.reshape(4, 2048, 4, 1024)
